# revision 24
# baseline (speedup 1.0000x reference)
"""Trainium2 Bass kernel for EASSA attention (8-core SPMD).

The reference module's state machine provably collapses: the create
score is `best - lam` with `lam = 1/max(budget, 1e-6) > 0`, so it can
never exceed `best` and a new state is created only when none exists
(t=0). A single state therefore accumulates the running mean of V, the
softmax over one valid state is exactly one-hot, and the attention
output is the cumulative mean of V. For the whole module:

    out[b, s, :] = (cumsum_s(x[b]) / (s+1)) @ (wv @ wo) + (bv @ wo + bo)

Q/K projections and the energy controller cannot affect the output.

Sharding: 8 lanes = (batch b in 0..3) x (sequence half h in 0..1),
uniform SPMD program. Cores owning a second half receive the first half
as input `xp` and fold its column-sum in as a scan prefix (first-half
cores receive zeros, keeping the program uniform).

v2: bf16 end to end. x/xp/out/W ship as bf16 (halving HBM bytes), the
folded projection weight W = wv @ wo is built on the host, and phase 2
(prefix fold + projection + store) is fused per 128-token block right
behind that block's local cumsum so stores stream while later blocks
load. xp is DMAd first (sync queue) so the global prefix is ready a few
us in; consts/W/xin ride the gpsimd SWDGE queue. All matmuls are bf16
(PSUM accumulates fp32); carry chains stay fp32 on DVE.

Per-block pipeline:
  4x matmul (local cumsum, feature-major, PSUM) -> DVE carry-table
  update (exact fp32) -> 4x fold+round copy PSUM->SBUF bf16 with the
  global prefix as per-partition bias (2 on ACT, 2 on DVE) -> 4x
  matmul (projection vs W, token-major PSUM) [+ rank-1 bias matmul] ->
  scale by 1/(s+1) during the PSUM->SBUF copy (alternating ACT/DVE) ->
  one store DMA per 4 blocks.
"""

from contextlib import ExitStack

import ml_dtypes
import numpy as np

import concourse.bacc as bacc
import concourse.tile as tile
from concourse import mybir
from concourse.bass_utils import run_bass_kernel_spmd

F32 = mybir.dt.float32
BF16 = mybir.dt.bfloat16
BF_NP = ml_dtypes.bfloat16
P = 128          # partitions / tokens per block
D = 512          # model dim
HALF = 2048      # tokens per core
NBLK = HALF // P # 16
NCH = D // P     # 4 feature chunks
N_CORES = 8
NQ = 4           # xin quads (4 blocks each)


def build_nc(with_bias=True):
    """Build the (uniform SPMD) Bass program for one core."""
    nc = bacc.Bacc("TRN2", target_bir_lowering=False, debug=False)

    xin = nc.dram_tensor("xin", [HALF, D], BF16, kind="ExternalInput").ap()
    xp = nc.dram_tensor("xp", [HALF, D], BF16, kind="ExternalInput").ap()
    w = nc.dram_tensor("w", [D, D], BF16, kind="ExternalInput").ap()
    u_tri = nc.dram_tensor("u_tri", [P, P], BF16, kind="ExternalInput").ap()
    ones_col = nc.dram_tensor("ones_col", [P, 1], BF16, kind="ExternalInput").ap()
    invs_cols = nc.dram_tensor("invs_cols", [P, NBLK], F32, kind="ExternalInput").ap()
    if with_bias:
        counts_row = nc.dram_tensor("counts_row", [1, HALF], BF16, kind="ExternalInput").ap()
        c_row = nc.dram_tensor("c_row", [1, D], BF16, kind="ExternalInput").ap()
    out = nc.dram_tensor("out", [HALF, D], BF16, kind="ExternalOutput").ap()

    with tile.TileContext(nc) as tc, ExitStack() as ctx:
        consts = ctx.enter_context(tc.tile_pool(name="consts", bufs=1))

        # sync HWDGE queue in wire-priority order: tiny consts, first
        # xin quad (unblocks the cumsum pipeline), xp (unblocks the
        # global prefix), remaining quads; stores are issued here later.
        # gpsimd SWDGE carries only W (its per-issue DRAIN is ~2.2us).
        u_sb = consts.tile([P, P], BF16, tag="u")
        nc.sync.dma_start(u_sb[:], u_tri[:])
        onec_sb = consts.tile([P, 1], BF16, tag="onec")
        nc.sync.dma_start(onec_sb[:], ones_col[:])
        invs_sb = consts.tile([P, NBLK], F32, tag="invs")
        nc.sync.dma_start(invs_sb[:], invs_cols[:])
        if with_bias:
            counts_sb = consts.tile([1, HALF], BF16, tag="counts")
            nc.sync.dma_start(counts_sb[:], counts_row[:])
            c_sb = consts.tile([1, D], BF16, tag="c")
            nc.sync.dma_start(c_sb[:], c_row[:])
        xin_pool = ctx.enter_context(tc.tile_pool(name="xin", bufs=1))
        xinv = xin.rearrange("(n p) d -> p n d", p=P)
        xq_tiles = []
        for qi in range(NQ):
            xq_tiles.append(
                xin_pool.tile([P, 4 * D], BF16, tag=f"xq{qi}", name=f"xq{qi}")
            )

        def load_xq(qi):
            nc.sync.dma_start(
                xq_tiles[qi][:].rearrange("p (n d) -> p n d", d=D),
                xinv[:, 4 * qi:4 * (qi + 1), :],
            )

        # xp first on the wire, as 4 separate tiles so each sub-quad's
        # partial tree-fold starts the moment that sub-DMA lands; then
        # xq0, then W, then the remaining xin quads.
        xpt = [
            consts.tile([P, 4 * D], BF16, tag=f"xp{si}", name=f"xp{si}")
            for si in range(4)
        ]
        xpv = xp.rearrange("(n p) d -> p n d", p=P)
        for si in range(4):
            nc.sync.dma_start(
                xpt[si][:].rearrange("p (n d) -> p n d", d=D),
                xpv[:, 4 * si:4 * (si + 1), :],
            )
        load_xq(0)
        w_sb = consts.tile([P, NCH * D], BF16, tag="w")
        nc.sync.dma_start(
            w_sb[:].rearrange("p (j m) -> p j m", m=D),
            w.rearrange("(j p) m -> p j m", p=P),
        )
        for qi in range(1, NQ):
            load_xq(qi)

        # per-sub-quad fold 4 rows -> 1 (DVE; the scalar engine has no
        # tensor_tensor), then 3 cross-quad adds.
        for si in range(4):
            t = xpt[si]
            nc.vector.tensor_add(
                t[:, 0:2 * D], t[:, 0:2 * D], t[:, 2 * D:4 * D]
            )
            nc.vector.tensor_add(t[:, 0:D], t[:, 0:D], t[:, D:2 * D])
        nc.vector.tensor_add(xpt[0][:, 0:D], xpt[0][:, 0:D], xpt[1][:, 0:D])
        nc.vector.tensor_add(xpt[2][:, 0:D], xpt[2][:, 0:D], xpt[3][:, 0:D])
        nc.vector.tensor_add(xpt[0][:, 0:D], xpt[0][:, 0:D], xpt[2][:, 0:D])

        pool_pp = ctx.enter_context(tc.tile_pool(name="psum_pp", bufs=1, space="PSUM"))
        pp = pool_pp.tile([P, NCH], F32, tag="pp")
        for j in range(NCH):
            nc.tensor.matmul(
                pp[:, j:j + 1],
                lhsT=xpt[0][:, j * P:(j + 1) * P],
                rhs=onec_sb[:],
                start=True,
                stop=True,
            )
        p_sb4 = consts.tile([P, NCH], F32, tag="p_sb4")
        nc.vector.tensor_copy(p_sb4[:], pp[:])

        # local carry table (exact fp32): p_all[:, 4b+j] = colsum of
        # blocks < b, chunk j; p_tot = p_all + global prefix.
        p_all = consts.tile([P, NCH * NBLK], F32, tag="p_all")
        p_tot = consts.tile([P, NCH * NBLK], F32, tag="p_tot")
        nc.vector.memset(p_all[:, 0:NCH], 0.0)

        psum_ct = ctx.enter_context(tc.tile_pool(name="psum_ct", bufs=4, space="PSUM"))
        psum_y = ctx.enter_context(tc.tile_pool(name="psum_y", bufs=3, space="PSUM"))
        cts_pool = ctx.enter_context(tc.tile_pool(name="cts", bufs=4))
        y_pool = ctx.enter_context(tc.tile_pool(name="y", bufs=2))

        outv = out.rearrange("(n p) d -> p n d", p=P)
        yq = None
        for blk in range(NBLK):
            xt = xq_tiles[blk // 4]
            xoff = (blk % 4) * D

            # feature-major local cumsum:
            # pct[:, j*128+s] = sum_{tau<=s} x[tau, j*128+f]
            pct = psum_ct.tile([P, D], F32, tag="pct")
            for j in range(NCH):
                nc.tensor.matmul(
                    pct[:, j * P:(j + 1) * P],
                    lhsT=xt[:, xoff + j * P:xoff + (j + 1) * P],
                    rhs=u_sb[:],
                    start=True,
                    stop=True,
                )
            # carry chain from PSUM last-token cols (exact fp32)
            if blk < NBLK - 1:
                nc.vector.tensor_add(
                    p_all[:, (blk + 1) * NCH:(blk + 2) * NCH],
                    p_all[:, blk * NCH:(blk + 1) * NCH],
                    pct[:, P - 1::P],
                )
            # p_tot on the (otherwise idle) Pool engine — SBUF-only op
            nc.gpsimd.tensor_add(
                p_tot[:, blk * NCH:(blk + 1) * NCH],
                p_all[:, blk * NCH:(blk + 1) * NCH],
                p_sb4[:],
            )

            # fold global prefix during the rounding PSUM->SBUF copy;
            # 3/1 vs 2/2 ACT/DVE split on alternating blocks balances
            # both engines at ~1.03us/block
            cts = cts_pool.tile([P, D], BF16, tag="cts")
            n_act = 3 if blk % 2 == 0 else 2
            for j in range(NCH):
                dst = cts[:, j * P:(j + 1) * P]
                src = pct[:, j * P:(j + 1) * P]
                sc = p_tot[:, blk * NCH + j:blk * NCH + j + 1]
                if j < n_act:
                    nc.scalar.add(dst, src, sc)
                else:
                    nc.vector.tensor_scalar_add(dst, src, sc)

            # projection: py[s, n] = sum_j cts_j[.., s].T @ W_j[.., n]
            py = psum_y.tile([P, D], F32, tag="py")
            for j in range(NCH):
                nc.tensor.matmul(
                    py[:],
                    lhsT=cts[:, j * P:(j + 1) * P],
                    rhs=w_sb[:, j * D:(j + 1) * D],
                    start=(j == 0),
                    stop=(j == NCH - 1) and not with_bias,
                )
            if with_bias:
                nc.tensor.matmul(
                    py[:],
                    lhsT=counts_sb[:, blk * P:(blk + 1) * P],
                    rhs=c_sb[:],
                    start=False,
                    stop=True,
                )

            # scale by 1/(s+1) during the PSUM->SBUF copy. Early blocks
            # scale on the idle Pool engine (they are prefix-gated and
            # their store trails by a quad anyway); late blocks split
            # ACT/DVE to keep the critical tail on the fast engines.
            if blk % 4 == 0:
                yq = y_pool.tile([P, 4 * D], BF16, tag="yq")
            ysl = yq[:, (blk % 4) * D:(blk % 4 + 1) * D]
            if blk % 2 == 0:
                nc.vector.tensor_scalar_mul(ysl, py[:], invs_sb[:, blk:blk + 1])
            else:
                nc.scalar.mul(ysl, py[:], invs_sb[:, blk:blk + 1])
            if blk % 4 == 3 and blk != NBLK - 1:
                qi = blk // 4
                nc.sync.dma_start(
                    outv[:, 4 * qi:4 * (qi + 1), :],
                    yq[:].rearrange("p (n d) -> p n d", d=D),
                )
            elif blk == NBLK - 2:
                # split the final quad's store so the tail is shorter
                nc.sync.dma_start(
                    outv[:, NBLK - 4:NBLK - 1, :],
                    yq[:, 0:3 * D].rearrange("p (n d) -> p n d", d=D),
                )
            elif blk == NBLK - 1:
                nc.sync.dma_start(
                    outv[:, NBLK - 1:NBLK, :],
                    yq[:, 3 * D:4 * D].rearrange("p (n d) -> p n d", d=D),
                )

    nc.compile()
    return nc


def make_in_maps(x, wv, bv, wo, bo, with_bias):
    B, S, Dm = x.shape
    assert (B, S, Dm) == (4, 4096, 512)
    x_bf = np.ascontiguousarray(np.asarray(x, dtype=np.float32)).astype(BF_NP)
    wv = np.asarray(wv, dtype=np.float32)
    wo = np.asarray(wo, dtype=np.float32)
    w_bf = np.ascontiguousarray((wv @ wo).astype(BF_NP))

    u_tri = np.triu(np.ones((P, P), dtype=np.float32)).astype(BF_NP)
    ones_col = np.ones((P, 1), dtype=np.float32).astype(BF_NP)
    zeros_half = np.zeros((HALF, D), dtype=BF_NP)

    in_maps = []
    for c in range(N_CORES):
        b, h = c // 2, c % 2
        off = h * HALF
        counts = np.arange(off + 1, off + HALF + 1, dtype=np.float32)
        im = {
            "xin": np.ascontiguousarray(x_bf[b, off:off + HALF, :]),
            "xp": np.ascontiguousarray(x_bf[b, 0:HALF, :]) if h == 1 else zeros_half,
            "w": w_bf,
            "u_tri": u_tri,
            "ones_col": ones_col,
            "invs_cols": np.ascontiguousarray((1.0 / counts).reshape(NBLK, P).T),
        }
        if with_bias:
            bv32 = np.asarray(bv, dtype=np.float32)
            bo32 = np.asarray(bo, dtype=np.float32)
            c_vec = (bv32 @ wo + bo32).astype(BF_NP)
            im["counts_row"] = np.ascontiguousarray(
                counts.astype(BF_NP).reshape(1, HALF))
            im["c_row"] = np.ascontiguousarray(c_vec.reshape(1, D))
        in_maps.append(im)
    return in_maps


_NC_CACHE = {}


def run(inputs, trace=False, trace_cores=None):
    """Shard, run on 8 cores, gather. Returns (out, BassKernelResults)."""
    with_bias = bool(
        np.any(np.asarray(inputs["bv"])) or np.any(np.asarray(inputs["bo"]))
    )
    key = ("nc", with_bias)
    if key not in _NC_CACHE:
        _NC_CACHE[key] = build_nc(with_bias=with_bias)
    nc = _NC_CACHE[key]
    in_maps = make_in_maps(
        inputs["x"], inputs["wv"], inputs["bv"], inputs["wo"], inputs["bo"],
        with_bias,
    )
    res = run_bass_kernel_spmd(
        nc, in_maps, list(range(N_CORES)), trace=trace, trace_cores=trace_cores
    )
    out = np.empty((4, 4096, 512), dtype=np.float32)
    for c in range(N_CORES):
        b, h = c // 2, c % 2
        out[b, h * HALF:(h + 1) * HALF, :] = np.asarray(
            res.results[c]["out"]).astype(np.float32)
    return out, res


def kernel(**inputs):
    out, _ = run(inputs, trace=False)
    return out
